# revision 2
# baseline (speedup 1.0000x reference)
"""AnyPrecisionLinear (4-bit LUT dequant + matmul) on 8 TRN2 NeuronCores.

y = x @ W.T with W[o,i] = lut[o, qweight[o,i]].

Sharding: column-parallel over out_features (1376 rows/core, padded to 1408).
Per core pipeline:
  - dequant: 8 custom fused DVE pair-ops per 128-row o-tile:
        acc' = acc + (q==k)*lut_k + (q==k+1)*lut_{k+1}
  - transpose W tiles on TensorE (identity matmul) -> PSUM -> ACT copy to SBUF
  - matmul: W.T stationary [128i x 128o], moving x.T [128i x 512b], PSUM fp32
  - y.T written fp16; host concatenates, slices padding, transposes back.

Self-contained: hardcodes all shapes; builds+compiles once per process.
"""

import re
import sys

sys.path.insert(0, "/opt/trn_rl_repo")

import numpy as np

import concourse.mybir as mybir
import concourse.tile as tile
from concourse import bacc
from concourse.masks import make_identity

FP16 = mybir.dt.float16
FP32 = mybir.dt.float32

NCORES = 8
IN = 4096
BATCH = 4096
OUT = 11008
OUT_SLICE = OUT // NCORES  # 1376
OTILES = 11
OUT_PAD = OTILES * 128  # 1408
NK = 16
BC = 512
NBC = BATCH // BC  # 8
ITILES = IN // 128  # 32
GROUPS = [1, 2, 4, 4]
ALU = mybir.AluOpType


def _register_pair_op():
    from concourse.dve_ops import (
        OPS,
        _SUB_OPCODE_FOR_NAME,
        _CUSTOM_DVE_ROW_BASE,
        CUSTOM_DVE_SPECS,
        DveOp,
    )
    from concourse.dve_spec import Spec, Src0, Src1, C0, C1, C2, One, eq

    name = "ANYPREC_PAIR_ANT"
    if name in _SUB_OPCODE_FOR_NAME:
        return next(op for op in OPS if op.name == name)

    body = (Src0 + eq(Src1, C2) * C0) + eq(Src1, C2 + One) * C1

    def _ref(in0, in1, s0, s1, imm2):
        dd = in1.astype(np.float32) - imm2
        return (
            in0.astype(np.float32)
            + (dd == 0.0) * np.asarray(s0, np.float32)
            + (dd == 1.0) * np.asarray(s1, np.float32)
        ).astype(np.float32)

    op = DveOp(name, Spec(body=body, reference=_ref), subdim=False, uops_sha={})
    _SUB_OPCODE_FOR_NAME[name] = _CUSTOM_DVE_ROW_BASE + len(OPS)
    OPS.append(op)
    CUSTOM_DVE_SPECS[name] = op.spec
    for ver in ("v3",):
        try:
            op.compile(ver)
        except ValueError as e:
            m = re.search(r"\(%s: ([0-9a-f]+) " % ver, str(e))
            if not m:
                raise
            op.uops_sha[ver] = m.group(1)
            op.compile(ver)
    return op


def _build():
    pair_op = _register_pair_op()
    nc = bacc.Bacc(None, target_bir_lowering=False, debug=False)
    xt_ext = nc.declare_dram_parameter("xt", [IN, BATCH], FP16, isOutput=False)
    qf_ext = nc.declare_dram_parameter("qf", [OUT_PAD, IN], FP16, isOutput=False)
    lut_ext = nc.declare_dram_parameter("lut", [OUT_PAD, NK], FP32, isOutput=False)
    yt_ext = nc.declare_dram_parameter("yt", [OUT_PAD, BATCH], FP16, isOutput=True)

    with tile.TileContext(nc) as tc:
        with (
            tc.tile_pool(name="const", bufs=1) as const_pool,
            tc.tile_pool(name="qp", bufs=2) as q_pool,
            tc.tile_pool(name="lutp", bufs=2) as lut_pool,
            tc.tile_pool(name="accp", bufs=4) as acc_pool,
            tc.tile_pool(name="wtp", bufs=8) as wt_pool,
            tc.tile_pool(name="xp", bufs=2) as x_pool,
            tc.tile_pool(name="ysp", bufs=3) as ys_pool,
            tc.tile_pool(name="tpp", bufs=4, space="PSUM") as tp_pool,
            tc.tile_pool(name="ypp", bufs=4, space="PSUM") as y_pool,
        ):
            ident = const_pool.tile([128, 128], FP16)
            make_identity(nc, ident[:])
            zeros = const_pool.tile([128, IN], FP16)
            nc.gpsimd.memset(zeros[:], 0.0)

            def dequant(og):
                q = q_pool.tile([128, IN], FP16, tag="q")
                half = IN // 2
                nc.sync.dma_start(
                    out=q[:, :half], in_=qf_ext[og * 128 : (og + 1) * 128, :half]
                )
                nc.sync.dma_start(
                    out=q[:, half:], in_=qf_ext[og * 128 : (og + 1) * 128, half:]
                )
                lt = lut_pool.tile([128, NK], FP32, tag="lt")
                nc.sync.dma_start(out=lt[:], in_=lut_ext[og * 128 : (og + 1) * 128, :])
                acc = zeros
                for p in range(8):
                    nacc = acc_pool.tile([128, IN], FP16, tag="acc")
                    nc.vector._custom_dve(
                        pair_op,
                        out=nacc[:],
                        in0=acc[:],
                        in1=q[:],
                        s0=lt[:, 2 * p : 2 * p + 1],
                        s1=lt[:, 2 * p + 1 : 2 * p + 2],
                        imm2=float(2 * p),
                    )
                    acc = nacc
                wt = wt_pool.tile([128, IN], FP16, tag="wt")
                for i0 in range(ITILES):
                    sl = slice(i0 * 128, (i0 + 1) * 128)
                    tp = tp_pool.tile([128, 128], FP16, tag="tp")
                    nc.tensor.transpose(tp[:], acc[:, sl], ident[:])
                    nc.scalar.copy(out=wt[:, sl], in_=tp[:])
                return wt

            og0 = 0
            for g in GROUPS:
                ogs = list(range(og0, og0 + g))
                og0 += g
                wts = [dequant(og) for og in ogs]
                for bc in range(NBC):
                    xb = x_pool.tile([128, ITILES * BC], FP16, tag="xb")
                    # xb[p, i0*BC + b] = xt[i0*128 + p, bc*BC + b]; 4 split DMAs
                    nsplit = 4
                    ichunk = ITILES // nsplit
                    for s in range(nsplit):
                        nc.sync.dma_start(
                            out=xb[:, s * ichunk * BC : (s + 1) * ichunk * BC].rearrange(
                                "p (i b) -> p i b", i=ichunk
                            ),
                            in_=xt_ext[
                                s * ichunk * 128 : (s + 1) * ichunk * 128,
                                bc * BC : (bc + 1) * BC,
                            ].rearrange("(i p) b -> p i b", p=128),
                        )
                    for og, wt in zip(ogs, wts):
                        yp = y_pool.tile([128, BC], FP32, tag="yp")
                        for i0 in range(ITILES):
                            nc.tensor.matmul(
                                yp[:],
                                lhsT=wt[:, i0 * 128 : (i0 + 1) * 128],
                                rhs=xb[:, i0 * BC : (i0 + 1) * BC],
                                start=(i0 == 0),
                                stop=(i0 == ITILES - 1),
                            )
                        ys = ys_pool.tile([128, BC], FP16, tag="ys")
                        nc.scalar.copy(out=ys[:], in_=yp[:])
                        nc.sync.dma_start(
                            out=yt_ext[og * 128 : (og + 1) * 128, bc * BC : (bc + 1) * BC],
                            in_=ys[:],
                        )
    nc.finalize()
    return nc


_STATE = {}


def _get_compiled():
    if "cb" in _STATE:
        return _STATE["cb"]
    import jax
    from jax.sharding import Mesh, PartitionSpec, NamedSharding
    from jax.experimental.shard_map import shard_map
    from concourse.bass2jax import (
        _bass_exec_p,
        install_neuronx_cc_hook,
        partition_id_tensor,
    )

    try:
        jax.config.update("jax_compilation_cache_dir", "/tmp/.anyprec_jaxcache")
        jax.config.update("jax_persistent_cache_min_compile_time_secs", 10)
        jax.config.update("jax_persistent_cache_min_entry_size_bytes", 0)
    except Exception:
        pass

    install_neuronx_cc_hook()
    nc = _build()

    partition_name = nc.partition_id_tensor.name if nc.partition_id_tensor else None
    in_names, out_names, out_avals = [], [], []
    for alloc in nc.m.functions[0].allocations:
        if not isinstance(alloc, mybir.MemoryLocationSet):
            continue
        name = alloc.memorylocations[0].name
        if alloc.kind == "ExternalInput":
            if name != partition_name:
                in_names.append(name)
        elif alloc.kind == "ExternalOutput":
            out_names.append(name)
            out_avals.append(
                jax.core.ShapedArray(tuple(alloc.tensor_shape), mybir.dt.np(alloc.dtype))
            )
    all_in_names = in_names + out_names
    if partition_name is not None:
        all_in_names.append(partition_name)

    def _body(*args):
        operands = list(args)
        if partition_name is not None:
            operands.append(partition_id_tensor())
        return tuple(
            _bass_exec_p.bind(
                *operands,
                out_avals=tuple(out_avals),
                in_names=tuple(all_in_names),
                out_names=tuple(out_names),
                lowering_input_output_aliases=(),
                sim_require_finite=True,
                sim_require_nnan=True,
                nc=nc,
            )
        )

    devices = jax.devices()[:NCORES]
    mesh = Mesh(np.asarray(devices), ("core",))
    nin = len(in_names) + len(out_names)
    fn = jax.jit(
        shard_map(
            _body,
            mesh=mesh,
            in_specs=(PartitionSpec("core"),) * nin,
            out_specs=(PartitionSpec("core"),) * len(out_names),
            check_rep=False,
        ),
        keep_unused=True,
    )
    cb = {
        "fn": fn,
        "in_names": in_names,
        "out_names": out_names,
        "out_avals": out_avals,
        "sharding": NamedSharding(mesh, PartitionSpec("core")),
        "jax": jax,
    }
    _STATE["cb"] = cb
    return cb


def prepare_inputs(x, lut, qweight):
    """Returns the concatenated (8*rows, ...) arrays for xt, qf, lut."""
    x = np.asarray(x)
    lut = np.asarray(lut)
    qweight = np.asarray(qweight)
    xt = np.ascontiguousarray(x.astype(np.float16).T)  # [IN, BATCH]
    qf_full = qweight.astype(np.float16)  # exact for 0..15
    lut_full = lut.astype(np.float32)

    xt_cat = np.concatenate([xt] * NCORES, axis=0)
    qf_cat = np.zeros((NCORES * OUT_PAD, IN), np.float16)
    lut_cat = np.zeros((NCORES * OUT_PAD, NK), np.float32)
    for c in range(NCORES):
        r0, r1 = c * OUT_SLICE, (c + 1) * OUT_SLICE
        qf_cat[c * OUT_PAD : c * OUT_PAD + OUT_SLICE] = qf_full[r0:r1]
        lut_cat[c * OUT_PAD : c * OUT_PAD + OUT_SLICE] = lut_full[r0:r1]
    return {"xt": xt_cat, "qf": qf_cat, "lut": lut_cat}


def run_device(arrs, bench_reps=0):
    cb = _get_compiled()
    jax = cb["jax"]
    dev_args = [
        jax.device_put(arrs[n], cb["sharding"]) for n in cb["in_names"]
    ] + [
        jax.device_put(
            np.zeros((NCORES * a.shape[0], *a.shape[1:]), a.dtype), cb["sharding"]
        )
        for a in cb["out_avals"]
    ]
    jax.block_until_ready(dev_args)
    outs = cb["fn"](*dev_args)
    jax.block_until_ready(outs)
    result = np.asarray(outs[0])  # [8*OUT_PAD, BATCH] fp16

    timing = None
    if bench_reps:
        import time

        def run_n(n):
            best = None
            for _ in range(2):
                t0 = time.perf_counter()
                o = None
                for _ in range(n):
                    o = cb["fn"](*dev_args)
                jax.block_until_ready(o)
                dt = time.perf_counter() - t0
                best = dt if best is None else min(best, dt)
            return best

        n1, n2 = 10, 10 + bench_reps
        t1, t2 = run_n(n1), run_n(n2)
        timing = (t2 - t1) / (n2 - n1)
    return result, timing


def kernel(x, lut, qweight, w_bits=4, _bench_reps=0):
    arrs = prepare_inputs(x, lut, qweight)
    yt_cat, timing = run_device(arrs, bench_reps=_bench_reps)
    yt = yt_cat.reshape(NCORES, OUT_PAD, BATCH)[:, :OUT_SLICE, :].reshape(OUT, BATCH)
    y = np.ascontiguousarray(yt.T)  # [BATCH, OUT] fp16
    if _bench_reps:
        kernel._last_timing = timing
    return y


# revision 3
# speedup vs baseline: 1.7753x; 1.7753x over previous
"""AnyPrecisionLinear (4-bit LUT dequant + matmul) on 8 TRN2 NeuronCores.

y = x @ W.T with W[o,i] = lut[o, qweight[o,i]].

Sharding: column-parallel over out_features (1376 rows/core, padded to 1408).
Per core pipeline (per 128-row o-tile):
  - dequant: 8 custom fused DVE pair-ops: acc' = acc + (q==k)*lut_k + (q==k+1)*lut_{k+1}
  - transpose W tiles (TensorE identity matmul -> PSUM -> ACT copy, or DMA xbar)
  - matmul: W.T stationary [128i x 128o], moving x.T [128i x 512b], PSUM fp32
  - y.T written fp16; host concatenates, slices padding, transposes back.

o-tiles are processed in progressive groups; each group's batch sweep streams
x once while the next group dequantizes (emission interleaved so Tile's
program-order scheduling overlaps DVE/PE/DMA).
"""

import re
import sys

sys.path.insert(0, "/opt/trn_rl_repo")

import numpy as np

import concourse.mybir as mybir
import concourse.tile as tile
from concourse import bacc
from concourse.masks import make_identity

FP16 = mybir.dt.float16
FP32 = mybir.dt.float32

NCORES = 8
IN = 4096
BATCH = 4096
OUT = 11008
OUT_SLICE = OUT // NCORES  # 1376
OTILES = 11
OUT_PAD = OTILES * 128  # 1408
NK = 16
BC = 512
NBC = BATCH // BC  # 8
ITILES = IN // 128  # 32
ALU = mybir.AluOpType

OPT = {
    "groups": [1, 2, 4, 4],
    "transpose": "pe",  # "pe" | "dma"
    "x_splits": 8,
}


def _register_pair_op():
    from concourse.dve_ops import (
        OPS,
        _SUB_OPCODE_FOR_NAME,
        _CUSTOM_DVE_ROW_BASE,
        CUSTOM_DVE_SPECS,
        DveOp,
    )
    from concourse.dve_spec import Spec, Src0, Src1, C0, C1, C2, One, eq

    name = "ANYPREC_PAIR_ANT"
    if name in _SUB_OPCODE_FOR_NAME:
        return next(op for op in OPS if op.name == name)

    body = (Src0 + eq(Src1, C2) * C0) + eq(Src1, C2 + One) * C1

    def _ref(in0, in1, s0, s1, imm2):
        dd = in1.astype(np.float32) - imm2
        return (
            in0.astype(np.float32)
            + (dd == 0.0) * np.asarray(s0, np.float32)
            + (dd == 1.0) * np.asarray(s1, np.float32)
        ).astype(np.float32)

    op = DveOp(name, Spec(body=body, reference=_ref), subdim=False, uops_sha={})
    _SUB_OPCODE_FOR_NAME[name] = _CUSTOM_DVE_ROW_BASE + len(OPS)
    OPS.append(op)
    CUSTOM_DVE_SPECS[name] = op.spec
    for ver in ("v3",):
        try:
            op.compile(ver)
        except ValueError as e:
            m = re.search(r"\(%s: ([0-9a-f]+) " % ver, str(e))
            if not m:
                raise
            op.uops_sha[ver] = m.group(1)
            op.compile(ver)
    return op


def _build(opt=None):
    opt = {**OPT, **(opt or {})}
    groups = opt["groups"]
    assert sum(groups) == OTILES
    pair_op = _register_pair_op()
    nc = bacc.Bacc(None, target_bir_lowering=False, debug=False)
    xt_ext = nc.declare_dram_parameter("xt", [IN, BATCH], FP16, isOutput=False)
    qf_ext = nc.declare_dram_parameter("qf", [OUT_PAD, IN], FP16, isOutput=False)
    lut_ext = nc.declare_dram_parameter("lut", [OUT_PAD, NK], FP32, isOutput=False)
    yt_ext = nc.declare_dram_parameter("yt", [OUT_PAD, BATCH], FP16, isOutput=True)

    with tile.TileContext(nc) as tc:
        with (
            tc.tile_pool(name="const", bufs=1) as const_pool,
            tc.tile_pool(name="qp", bufs=2) as q_pool,
            tc.tile_pool(name="lutp", bufs=2) as lut_pool,
            tc.tile_pool(name="accp", bufs=3) as acc_pool,
            tc.tile_pool(name="wtp", bufs=8) as wt_pool,
            tc.tile_pool(name="xp", bufs=2) as x_pool,
            tc.tile_pool(name="ysp", bufs=3) as ys_pool,
            tc.tile_pool(name="tpp", bufs=4, space="PSUM") as tp_pool,
            tc.tile_pool(name="ypp", bufs=4, space="PSUM") as y_pool,
        ):
            ident = const_pool.tile([128, 128], FP16)
            make_identity(nc, ident[:])
            zeros = const_pool.tile([128, IN], FP16)
            nc.vector.memset(zeros[:], 0.0)

            def dequant(og):
                """Emit dequant chain + transposes for o-tile og; returns wt."""
                q = q_pool.tile([128, IN], FP16, tag="q")
                half = IN // 2
                nc.sync.dma_start(
                    out=q[:, :half], in_=qf_ext[og * 128 : (og + 1) * 128, :half]
                )
                nc.sync.dma_start(
                    out=q[:, half:], in_=qf_ext[og * 128 : (og + 1) * 128, half:]
                )
                lt = lut_pool.tile([128, NK], FP32, tag="lt")
                nc.sync.dma_start(out=lt[:], in_=lut_ext[og * 128 : (og + 1) * 128, :])
                acc = zeros
                for p in range(8):
                    nacc = acc_pool.tile([128, IN], FP16, tag="acc")
                    nc.vector._custom_dve(
                        pair_op,
                        out=nacc[:],
                        in0=acc[:],
                        in1=q[:],
                        s0=lt[:, 2 * p : 2 * p + 1],
                        s1=lt[:, 2 * p + 1 : 2 * p + 2],
                        imm2=float(2 * p),
                    )
                    acc = nacc
                wt = wt_pool.tile([128, IN], FP16, tag="wt")
                for i0 in range(ITILES):
                    sl = slice(i0 * 128, (i0 + 1) * 128)
                    if opt["transpose"] == "pe":
                        tp = tp_pool.tile([128, 128], FP16, tag="tp")
                        nc.tensor.transpose(tp[:], acc[:, sl], ident[:])
                        nc.scalar.copy(out=wt[:, sl], in_=tp[:])
                    else:
                        nc.scalar.dma_start_transpose(out=wt[:, sl], in_=acc[:, sl])
                return wt

            def emit_xblock(bc):
                xb = x_pool.tile([128, ITILES * BC], FP16, tag="xb")
                nsplit = opt["x_splits"]
                ichunk = ITILES // nsplit
                for s in range(nsplit):
                    nc.gpsimd.dma_start(
                        out=xb[:, s * ichunk * BC : (s + 1) * ichunk * BC].rearrange(
                            "p (i b) -> p i b", i=ichunk
                        ),
                        in_=xt_ext[
                            s * ichunk * 128 : (s + 1) * ichunk * 128,
                            bc * BC : (bc + 1) * BC,
                        ].rearrange("(i p) b -> p i b", p=128),
                    )
                return xb

            def emit_mm(og, wt, bc, xb):
                yp = y_pool.tile([128, BC], FP32, tag="yp")
                for i0 in range(ITILES):
                    nc.tensor.matmul(
                        yp[:],
                        lhsT=wt[:, i0 * 128 : (i0 + 1) * 128],
                        rhs=xb[:, i0 * BC : (i0 + 1) * BC],
                        start=(i0 == 0),
                        stop=(i0 == ITILES - 1),
                    )
                ys = ys_pool.tile([128, BC], FP16, tag="ys")
                nc.scalar.copy(out=ys[:], in_=yp[:])
                nc.scalar.dma_start(
                    out=yt_ext[og * 128 : (og + 1) * 128, bc * BC : (bc + 1) * BC],
                    in_=ys[:],
                )

            og0 = 0
            wts = {}
            for gi, g in enumerate(groups):
                ogs = list(range(og0, og0 + g))
                og0 += g
                if gi == 0:
                    for og in ogs:
                        wts[og] = dequant(og)
                nxt = (
                    list(range(og0, og0 + groups[gi + 1]))
                    if gi + 1 < len(groups)
                    else []
                )
                # interleave next group's dequants between this group's bc chunks
                for bc in range(NBC):
                    if bc < len(nxt):
                        wts[nxt[bc]] = dequant(nxt[bc])
                    xb = emit_xblock(bc)
                    for og in ogs:
                        emit_mm(og, wts[og], bc, xb)
                for og in ogs:
                    del wts[og]
    nc.finalize()
    return nc


_STATE = {}


def _get_compiled(opt=None):
    if "cb" in _STATE:
        return _STATE["cb"]
    import jax
    from jax.sharding import Mesh, PartitionSpec, NamedSharding
    from jax.experimental.shard_map import shard_map
    from concourse.bass2jax import (
        _bass_exec_p,
        install_neuronx_cc_hook,
        partition_id_tensor,
    )

    try:
        jax.config.update("jax_compilation_cache_dir", "/tmp/.anyprec_jaxcache")
        jax.config.update("jax_persistent_cache_min_compile_time_secs", 10)
        jax.config.update("jax_persistent_cache_min_entry_size_bytes", 0)
    except Exception:
        pass

    install_neuronx_cc_hook()
    nc = _build(opt)

    partition_name = nc.partition_id_tensor.name if nc.partition_id_tensor else None
    in_names, out_names, out_avals = [], [], []
    for alloc in nc.m.functions[0].allocations:
        if not isinstance(alloc, mybir.MemoryLocationSet):
            continue
        name = alloc.memorylocations[0].name
        if alloc.kind == "ExternalInput":
            if name != partition_name:
                in_names.append(name)
        elif alloc.kind == "ExternalOutput":
            out_names.append(name)
            out_avals.append(
                jax.core.ShapedArray(tuple(alloc.tensor_shape), mybir.dt.np(alloc.dtype))
            )
    all_in_names = in_names + out_names
    if partition_name is not None:
        all_in_names.append(partition_name)

    def _body(*args):
        operands = list(args)
        if partition_name is not None:
            operands.append(partition_id_tensor())
        return tuple(
            _bass_exec_p.bind(
                *operands,
                out_avals=tuple(out_avals),
                in_names=tuple(all_in_names),
                out_names=tuple(out_names),
                lowering_input_output_aliases=(),
                sim_require_finite=True,
                sim_require_nnan=True,
                nc=nc,
            )
        )

    devices = jax.devices()[:NCORES]
    mesh = Mesh(np.asarray(devices), ("core",))
    nin = len(in_names) + len(out_names)
    fn = jax.jit(
        shard_map(
            _body,
            mesh=mesh,
            in_specs=(PartitionSpec("core"),) * nin,
            out_specs=(PartitionSpec("core"),) * len(out_names),
            check_rep=False,
        ),
        keep_unused=True,
    )
    cb = {
        "fn": fn,
        "in_names": in_names,
        "out_names": out_names,
        "out_avals": out_avals,
        "sharding": NamedSharding(mesh, PartitionSpec("core")),
        "jax": jax,
    }
    _STATE["cb"] = cb
    return cb


def prepare_inputs(x, lut, qweight):
    x = np.asarray(x)
    lut = np.asarray(lut)
    qweight = np.asarray(qweight)
    xt = np.ascontiguousarray(x.astype(np.float16).T)  # [IN, BATCH]
    qf_full = qweight.astype(np.float16)  # exact for 0..15
    lut_full = lut.astype(np.float32)

    xt_cat = np.concatenate([xt] * NCORES, axis=0)
    qf_cat = np.zeros((NCORES * OUT_PAD, IN), np.float16)
    lut_cat = np.zeros((NCORES * OUT_PAD, NK), np.float32)
    for c in range(NCORES):
        r0, r1 = c * OUT_SLICE, (c + 1) * OUT_SLICE
        qf_cat[c * OUT_PAD : c * OUT_PAD + OUT_SLICE] = qf_full[r0:r1]
        lut_cat[c * OUT_PAD : c * OUT_PAD + OUT_SLICE] = lut_full[r0:r1]
    return {"xt": xt_cat, "qf": qf_cat, "lut": lut_cat}


def run_device(arrs, bench_reps=0, opt=None):
    cb = _get_compiled(opt)
    jax = cb["jax"]
    dev_args = [jax.device_put(arrs[n], cb["sharding"]) for n in cb["in_names"]] + [
        jax.device_put(
            np.zeros((NCORES * a.shape[0], *a.shape[1:]), a.dtype), cb["sharding"]
        )
        for a in cb["out_avals"]
    ]
    jax.block_until_ready(dev_args)
    outs = cb["fn"](*dev_args)
    jax.block_until_ready(outs)
    result = np.asarray(outs[0])  # [8*OUT_PAD, BATCH] fp16

    timing = None
    if bench_reps:
        import time

        def run_n(n):
            best = None
            for _ in range(2):
                t0 = time.perf_counter()
                o = None
                for _ in range(n):
                    o = cb["fn"](*dev_args)
                jax.block_until_ready(o)
                dt = time.perf_counter() - t0
                best = dt if best is None else min(best, dt)
            return best

        n1, n2 = 10, 10 + bench_reps
        t1, t2 = run_n(n1), run_n(n2)
        timing = (t2 - t1) / (n2 - n1)
    return result, timing


def kernel(x, lut, qweight, w_bits=4, _bench_reps=0, _opt=None):
    arrs = prepare_inputs(x, lut, qweight)
    yt_cat, timing = run_device(arrs, bench_reps=_bench_reps, opt=_opt)
    yt = yt_cat.reshape(NCORES, OUT_PAD, BATCH)[:, :OUT_SLICE, :].reshape(OUT, BATCH)
    y = np.ascontiguousarray(yt.T)  # [BATCH, OUT] fp16
    if _bench_reps:
        kernel._last_timing = timing
    return y
